# revision 1
# baseline (speedup 1.0000x reference)
"""Trainium2 Bass kernel for nn_DiffForest (soft decision forest forward).

Math: per tree t, z = x @ w_d[t]; p = sigmoid(z); leaf path probs are products
of 8 factors p/(1-p) down a depth-8 tree; output = sum_t leaf_prob @ softmax(w_l[t]) / 10.

Kernel formulation (all on device except small weight prep):
  - The 512 "leaves" come in identical pairs -> fold to 256 paths; fold the
    pair-sum + 1/n_trees into the leaf weight matrix w2 (host, exact).
  - Path products move to log space:  -log P[q] = sum_path softplus(-z) + sum_{branch=1} z
    which is a matmul with a constant 0/1 matrix S [512, 256]:
        A = S^T @ [softplus(-z); z],   leaf_prob^T = exp(-A)   ([256 paths, batch])
    softplus(-z) = ln(1 + exp(-z)) via the Exp/Ln activation tables (one table set).
  - This leaves three matmul stages (decision matmul in bf16, S-matmul in fp32r,
    leaf matmul in bf16) with the contraction dim on partitions throughout; no
    on-device transposes are needed because the S-matmul naturally produces
    leaf-major layout.
  - Sharding: data-parallel over batch; each of the 8 cores takes 2048 rows of x,
    weights replicated, no collectives.
"""

import numpy as np
import ml_dtypes

import concourse.bacc as bacc
import concourse.mybir as mybir
import concourse.tile as tile
from concourse.tile import add_dep_helper
from concourse.bass_utils import run_bass_kernel_spmd

N_CORES = 8
BATCH = 16384
B_LOC = BATCH // N_CORES        # 2048 rows per core
IN_DIM = 2048
N_TREES = 10
ND_PAD = 256                    # decision nodes padded 255 -> 256
NQ = 256                        # folded path (leaf) count
CLASSES = 1000
CHUNK = 512                     # batch columns processed per chunk
KI = IN_DIM // 128              # 16 contraction tiles for the decision matmul

BF16 = mybir.dt.bfloat16
F32 = mybir.dt.float32
F32R = mybir.dt.float32r
F16 = mybir.dt.float16
AF = mybir.ActivationFunctionType

import os

_CACHE = {}



def _build(b_loc=B_LOC, n_trees=N_TREES):
    n_chunks = b_loc // CHUNK
    nc = bacc.Bacc("TRN2", target_bir_lowering=False)
    xt = nc.dram_tensor("xt", (IN_DIM, b_loc), BF16, kind="ExternalInput")
    wd = nc.dram_tensor("wd", (n_trees, IN_DIM, ND_PAD), BF16, kind="ExternalInput")
    smat = nc.dram_tensor("smat", (512, NQ), F32R, kind="ExternalInput")
    w2 = nc.dram_tensor("w2", (n_trees, NQ, CLASSES), BF16, kind="ExternalInput")
    out = nc.dram_tensor("out", (b_loc, CLASSES), F32, kind="ExternalOutput")

    with tile.TileContext(nc) as tc:
        with (
            tc.tile_pool(name="const", bufs=1) as constp,
            tc.tile_pool(name="sb", bufs=2) as sb,
            tc.tile_pool(name="wdp", bufs=3) as wdp,
            tc.tile_pool(name="ep", bufs=6) as ep,
            tc.tile_pool(name="gp", bufs=6) as gp,
            tc.tile_pool(name="outp", bufs=2) as outp,
            tc.tile_pool(name="lptp", bufs=1) as lptp,
            tc.tile_pool(name="pz", bufs=2, space="PSUM") as pzp,
            tc.tile_pool(name="plp", bufs=2, space="PSUM") as plpp,
            tc.tile_pool(name="po", bufs=2, space="PSUM") as pop,
        ):
            smat_sb = constp.tile([128, 4, NQ], F32R)
            w2_sb = constp.tile([128, n_trees, 2, CLASSES], BF16)

            GROUP = 5
            first_mm = [None]
            started = False

            def emit_mm2(ci, lpT):
                c0 = ci * CHUNK
                for s in range(CHUNK // 128):
                    po = pop.tile([128, 1024], F32, tag="po")
                    n_acc = n_trees * 2
                    i = 0
                    for t in range(n_trees):
                        for lt in range(2):
                            first = i == 0
                            last = i == n_acc - 1
                            lhsT = lpT[:, t, lt, s * 128 : (s + 1) * 128]
                            nc.tensor.matmul(
                                po[:, 0:500], lhsT, w2_sb[:, t, lt, 0:500],
                                start=first, stop=last,
                            )
                            nc.tensor.matmul(
                                po[:, 512:1012], lhsT, w2_sb[:, t, lt, 500:1000],
                                start=first, stop=last,
                            )
                            i += 1
                    osb = outp.tile([128, CLASSES], F32, tag="osb")
                    nc.vector.tensor_copy(osb[:, 0:500], po[:, 0:500])
                    nc.vector.tensor_copy(osb[:, 500:1000], po[:, 512:1012])
                    nc.sync.dma_start(
                        out[c0 + s * 128 : c0 + (s + 1) * 128, :], osb[:, :]
                    )

            for ci in range(n_chunks):
                c0 = ci * CHUNK
                xt_pieces = []
                for kq in range(4):
                    xp = sb.tile([128, 4, CHUNK], BF16, tag=f"xt{kq}")
                    xdma = nc.sync.dma_start(
                        xp[:, :, :],
                        xt[
                            4 * kq * 128 : 4 * (kq + 1) * 128, c0 : c0 + CHUNK
                        ].rearrange("(k p) n -> p k n", p=128),
                    )
                    xt_pieces.append(xp)
                lpT = lptp.tile([128, n_trees, 2, CHUNK], BF16, tag="lpT")
                for t0 in range(0, n_trees, GROUP):
                    group = list(range(t0, min(t0 + GROUP, n_trees)))
                    group_G = {}
                    group_E = {}
                    last_exp = None
                    for t in group:
                        wd_pieces = None
                        wd_sb = None
                        if ci == 0 and t == 0:
                            # first tree: 4 piece tiles -> the first matmul
                            # only waits on one 256KB piece + one xt piece
                            wd_pieces = []
                            for kq in range(4):
                                wp = constp.tile(
                                    [128, 4, ND_PAD], BF16, tag=f"wd0p{kq}"
                                )
                                wdma = nc.sync.dma_start(
                                    wp[:, :, :],
                                    wd[
                                        t, 4 * kq * 128 : 4 * (kq + 1) * 128, :
                                    ].rearrange("(k p) d -> p k d", p=128),
                                )
                                wd_pieces.append(wp)
                        else:
                            wd_sb = wdp.tile([128, KI, ND_PAD], BF16, tag="wd")
                            wd_dma = nc.sync.dma_start(
                                wd_sb[:, :, :],
                                wd[t, :, :].rearrange("(k p) d -> p k d", p=128),
                            )
                            if ci == 0 and t in (1, 2):
                                add_dep_helper(
                                    wd_dma.ins, first_mm[0].ins, sync=True,
                                    reason="startup: critical pieces first",
                                )
                        G = gp.tile([128, 4, CHUNK], F32R, tag="G")
                        E = ep.tile([128, 2, CHUNK], F16, tag="E")
                        group_G[t] = G
                        group_E[t] = E
                        for dt_ in range(2):
                            psz = pzp.tile([128, CHUNK], F32, tag="psz")
                            for k in range(KI):
                                if wd_sb is None:
                                    lhsT = wd_pieces[k // 4][
                                        :, k % 4, dt_ * 128 : (dt_ + 1) * 128
                                    ]
                                else:
                                    lhsT = wd_sb[:, k, dt_ * 128 : (dt_ + 1) * 128]
                                mm = nc.tensor.matmul(
                                    psz[:, :],
                                    lhsT,
                                    xt_pieces[k // 4][:, k % 4, :],
                                    start=(k == 0),
                                    stop=(k == KI - 1),
                                )
                                if first_mm[0] is None:
                                    first_mm[0] = mm
                            # Exp heads the ACT critical chain; CAST second
                            last_exp = nc.scalar.activation(
                                E[:, dt_, :], psz[:, :], AF.Exp, scale=-1.0
                            )
                            nc.vector.tensor_copy(G[:, 2 + dt_, :], psz[:, :])
                    # softplus(-z) = ln(exp(-z)+1); gate Lns on the group's
                    # last Exp to batch ACT table sets
                    for t in group:
                        for dt_ in range(2):
                            ln = nc.scalar.activation(
                                group_G[t][:, dt_, :],
                                group_E[t][:, dt_, :],
                                AF.Ln,
                                bias=1.0,
                            )
                            add_dep_helper(
                                ln.ins, last_exp.ins, sync=False,
                                reason="batch ACT Ln block after Exp block",
                            )
                    if not started:
                        nc.sync.dma_start(
                            smat_sb[:, :, :],
                            smat[:, :].rearrange("(k p) q -> p k q", p=128),
                        )
                        for t in range(n_trees):
                            nc.sync.dma_start(
                                w2_sb[:, t, :, :],
                                w2[t, :, :].rearrange("(l p) c -> p l c", p=128),
                            )
                        started = True
                    for t in group:
                        for lt in range(2):
                            plp = plpp.tile([128, CHUNK], F32, tag="plp")
                            for k in range(4):
                                nc.tensor.matmul(
                                    plp[:, :],
                                    smat_sb[:, k, lt * 128 : (lt + 1) * 128],
                                    group_G[t][:, k, :],
                                    start=(k == 0),
                                    stop=(k == 3),
                                )
                            nc.scalar.activation(
                                lpT[:, t, lt, :], plp[:, :], AF.Exp, scale=-1.0
                            )
                emit_mm2(ci, lpT)
    nc.compile()
    return nc


def _smat_np():
    S = np.zeros((512, NQ), np.float32)
    q = np.arange(NQ)
    for n in range(8):
        node = (2**n - 1) + (q >> (8 - n))
        branch = (q >> (7 - n)) & 1
        S[node, q] += 1.0
        S[256 + node, q] += branch.astype(np.float32)
    return S


def _prep_weights(w_d, w_l, n_trees=N_TREES):
    bf16 = ml_dtypes.bfloat16
    w_l = np.asarray(w_l, dtype=np.float32)
    m = w_l.max(axis=-1, keepdims=True)
    e = np.exp(w_l - m, dtype=np.float32)
    sm = e / e.sum(axis=-1, keepdims=True)
    w2 = ((sm[:, 0::2, :] + sm[:, 1::2, :]) * np.float32(1.0 / n_trees)).astype(bf16)
    wd_p = np.zeros((n_trees, IN_DIM, ND_PAD), np.float32)
    wd_p[:, :, : w_d.shape[2]] = w_d
    return wd_p.astype(bf16), _smat_np(), w2


last_bass_results = None


def kernel(x, w_d, w_l):
    global last_bass_results
    x = np.asarray(x)
    wd_bf, S, w2 = _prep_weights(np.asarray(w_d), np.asarray(w_l))
    x_bf = x.astype(ml_dtypes.bfloat16)
    in_maps = []
    for c in range(N_CORES):
        xt = np.ascontiguousarray(x_bf[c * B_LOC : (c + 1) * B_LOC, :].T)
        in_maps.append({"xt": xt, "wd": wd_bf, "smat": S, "w2": w2})
    if "nc" not in _CACHE:
        _CACHE["nc"] = _build()
    res = run_bass_kernel_spmd(_CACHE["nc"], in_maps, core_ids=list(range(N_CORES)))
    last_bass_results = res
    return np.concatenate([res.results[c]["out"] for c in range(N_CORES)], axis=0)



# revision 2
# speedup vs baseline: 1.0447x; 1.0447x over previous
"""Trainium2 Bass kernel for nn_DiffForest (soft decision forest forward).

Math: per tree t, z = x @ w_d[t]; p = sigmoid(z); leaf path probs are products
of 8 factors p/(1-p) down a depth-8 tree; output = sum_t leaf_prob @ softmax(w_l[t]) / 10.

Kernel formulation (all on device except small weight prep):
  - The 512 "leaves" come in identical pairs -> fold to 256 paths; fold the
    pair-sum + 1/n_trees into the leaf weight matrix w2 (host, exact).
  - Depth-7 split: path products go to log space only for the first 7 levels:
        C7[q7] = sum_{n<7} softplus(-z_node) + sum_{branch=1} z_node
    which is a matmul with a constant 0/1 matrix S7 [256, 128] (contraction
    [sp(-z); z] over the 127 internal nodes, 2 accumulating matmuls).
    The 8th level is handled elementwise:
        leaf_prob[2*q7]   = exp(-C7[q7]) * sigmoid(z_leaf[q7])
        leaf_prob[2*q7+1] = exp(-C7[q7]) * (1 - sigmoid(z_leaf[q7]))
    with sigmoid(z) = exp(-softplus(-z)) reusing the same Exp/Ln tables.
    This cuts stage-2 PE time 4x vs a full [512, 256] S-matmul.
  - Host permutes w_d columns so cols 0..126 are the internal nodes and cols
    128..255 are the level-7 (leaf-split) nodes, partition-major for
    contiguous DMA. w2 rows are split even/odd with the odd block negated so
    Q1 = (sigmoid - 1) * P7 feeds the leaf matmul directly.
  - Three matmul stages (decision bf16, S7 bf16, leaf bf16), contraction on
    partitions throughout; no on-device transposes.
  - Sharding: data-parallel over batch; each of the 8 cores takes 2048 rows
    of x, weights replicated, no collectives.
  - DMA issue spread across queues: x on Sync, w_d on Scalar (both HWDGE),
    w2/smat/out on GpSimd, so the startup critical path isn't serialized
    behind one queue.
"""

import numpy as np
import ml_dtypes

import concourse.bacc as bacc
import concourse.mybir as mybir
import concourse.tile as tile
from concourse.tile import add_dep_helper
from concourse.bass_utils import run_bass_kernel_spmd

N_CORES = 8
BATCH = 16384
B_LOC = BATCH // N_CORES        # 2048 rows per core
IN_DIM = 2048
N_TREES = 10
ND_PAD = 256                    # decision node columns after permute+pad
CLASSES = 1000
CHUNK = 512                     # batch columns processed per chunk
KI = IN_DIM // 128              # 16 contraction tiles for the decision matmul

BF16 = mybir.dt.bfloat16
F32 = mybir.dt.float32
F16 = mybir.dt.float16
AF = mybir.ActivationFunctionType
ALU = mybir.AluOpType

_CACHE = {}


def _build(b_loc=B_LOC, n_trees=N_TREES):
    n_chunks = b_loc // CHUNK
    nc = bacc.Bacc("TRN2", target_bir_lowering=False)
    xt = nc.dram_tensor("xt", (128, KI, b_loc), BF16, kind="ExternalInput")
    wd = nc.dram_tensor("wd", (n_trees, 128, KI, ND_PAD), BF16, kind="ExternalInput")
    smat = nc.dram_tensor("smat", (2, 128, 128), BF16, kind="ExternalInput")
    w2 = nc.dram_tensor("w2", (n_trees, 2, 128, CLASSES), BF16, kind="ExternalInput")
    out = nc.dram_tensor("out", (b_loc, CLASSES), F32, kind="ExternalOutput")

    with tile.TileContext(nc) as tc:
        with (
            tc.tile_pool(name="const", bufs=1) as constp,
            tc.tile_pool(name="sb", bufs=2) as sb,
            tc.tile_pool(name="wdp", bufs=3) as wdp,
            tc.tile_pool(name="ep", bufs=6) as ep,
            tc.tile_pool(name="gp", bufs=6) as gp,
            tc.tile_pool(name="s1p", bufs=6) as s1p,
            tc.tile_pool(name="sgp", bufs=4) as sgp,
            tc.tile_pool(name="qp", bufs=2) as qp,
            tc.tile_pool(name="outp", bufs=2) as outp,
            tc.tile_pool(name="pz", bufs=2, space="PSUM") as pzp,
            tc.tile_pool(name="pc", bufs=2, space="PSUM") as pcp,
            tc.tile_pool(name="po", bufs=2, space="PSUM") as pop,
        ):
            smat_sb = constp.tile([128, 2, 128], BF16)
            w2_sb = constp.tile([128, n_trees, 2, CLASSES], BF16)

            GROUP = 5
            first_mm = [None]
            started = False

            def emit_mm2(ci, Qt):
                c0 = ci * CHUNK
                for s in range(CHUNK // 128):
                    po = pop.tile([128, 1024], F32, tag="po")
                    n_acc = n_trees * 2
                    i = 0
                    for t in range(n_trees):
                        for lt in range(2):
                            first = i == 0
                            last = i == n_acc - 1
                            lhsT = Qt[:, t, lt, s * 128 : (s + 1) * 128]
                            nc.tensor.matmul(
                                po[:, 0:500], lhsT, w2_sb[:, t, lt, 0:500],
                                start=first, stop=last,
                            )
                            nc.tensor.matmul(
                                po[:, 512:1012], lhsT, w2_sb[:, t, lt, 500:1000],
                                start=first, stop=last,
                            )
                            i += 1
                    osb = outp.tile([128, CLASSES], F32, tag="osb")
                    nc.vector.tensor_copy(osb[:, 0:500], po[:, 0:500])
                    nc.vector.tensor_copy(osb[:, 500:1000], po[:, 512:1012])
                    nc.gpsimd.dma_start(
                        out[c0 + s * 128 : c0 + (s + 1) * 128, :], osb[:, :]
                    )

            for ci in range(n_chunks):
                c0 = ci * CHUNK
                xts = []
                for h in range(2):
                    xp = sb.tile([128, 8, CHUNK], BF16, tag=f"xt{h}")
                    if ci == 0 and h == 0:
                        # sliver the first piece so the first matmul only
                        # waits on 128KB + 64KB of DMA
                        nc.sync.dma_start(xp[:, 0:1, :], xt[:, 0:1, c0 : c0 + CHUNK])
                        nc.sync.dma_start(xp[:, 1:4, :], xt[:, 1:4, c0 : c0 + CHUNK])
                        nc.sync.dma_start(xp[:, 4:8, :], xt[:, 4:8, c0 : c0 + CHUNK])
                    else:
                        nc.sync.dma_start(
                            xp[:, :, :], xt[:, 8 * h : 8 * (h + 1), c0 : c0 + CHUNK]
                        )
                    xts.append(xp)
                Qt = qp.tile([128, n_trees, 2, CHUNK], BF16, tag="Q")
                for t0 in range(0, n_trees, GROUP):
                    group = list(range(t0, min(t0 + GROUP, n_trees)))
                    gG = {}
                    gE = {}
                    gS1 = {}
                    last_exp = None
                    for t in group:
                        wd_sb = wdp.tile([128, KI, ND_PAD], BF16, tag="wd")
                        if ci == 0 and t == 0:
                            nc.scalar.dma_start(wd_sb[:, 0:1, :], wd[t, :, 0:1, :])
                            nc.scalar.dma_start(wd_sb[:, 1:4, :], wd[t, :, 1:4, :])
                            nc.scalar.dma_start(wd_sb[:, 4:16, :], wd[t, :, 4:16, :])
                        else:
                            wd_dma = nc.scalar.dma_start(wd_sb[:, :, :], wd[t, :, :, :])
                            if ci == 0 and t in (1, 2):
                                add_dep_helper(
                                    wd_dma.ins, first_mm[0].ins, sync=True,
                                    reason="startup: critical pieces first",
                                )
                        G = gp.tile([128, 2, CHUNK], BF16, tag="G")
                        E = ep.tile([128, 2, CHUNK], F16, tag="E")
                        S1 = s1p.tile([128, CHUNK], BF16, tag="S1")
                        gG[t] = G
                        gE[t] = E
                        gS1[t] = S1
                        for dt_ in range(2):
                            psz = pzp.tile([128, CHUNK], F32, tag="psz")
                            for k in range(KI):
                                lhsT = wd_sb[:, k, dt_ * 128 : (dt_ + 1) * 128]
                                mm = nc.tensor.matmul(
                                    psz[:, :], lhsT, xts[k // 8][:, k % 8, :],
                                    start=(k == 0), stop=(k == KI - 1),
                                )
                                if first_mm[0] is None:
                                    first_mm[0] = mm
                            last_exp = nc.scalar.activation(
                                E[:, dt_, :], psz[:, :], AF.Exp, scale=-1.0
                            )
                            if dt_ == 0:
                                nc.vector.tensor_copy(G[:, 1, :], psz[:, :])
                    # softplus(-z) = ln(exp(-z)+1); gate Lns on the group's
                    # last Exp to batch ACT table sets
                    last_ln = None
                    for t in group:
                        ln0 = nc.scalar.activation(
                            gG[t][:, 0, :], gE[t][:, 0, :], AF.Ln, bias=1.0
                        )
                        add_dep_helper(
                            ln0.ins, last_exp.ins, sync=False,
                            reason="batch ACT Ln block after Exp",
                        )
                        ln1 = nc.scalar.activation(
                            gS1[t][:, :], gE[t][:, 1, :], AF.Ln, bias=1.0
                        )
                        add_dep_helper(
                            ln1.ins, last_exp.ins, sync=False,
                            reason="batch ACT Ln block after Exp",
                        )
                        last_ln = ln1
                    if not started:
                        nc.gpsimd.dma_start(
                            smat_sb[:, :, :],
                            smat[:, :, :].rearrange("j p q -> p j q"),
                        )
                        for t in range(n_trees):
                            nc.gpsimd.dma_start(
                                w2_sb[:, t, :, :],
                                w2[t, :, :, :].rearrange("l p c -> p l c"),
                            )
                        started = True
                    for t in group:
                        psC = pcp.tile([128, CHUNK], F32, tag="psC")
                        nc.tensor.matmul(
                            psC[:, :], smat_sb[:, 0, :], gG[t][:, 0, :],
                            start=True, stop=False,
                        )
                        nc.tensor.matmul(
                            psC[:, :], smat_sb[:, 1, :], gG[t][:, 1, :],
                            start=False, stop=True,
                        )
                        PS = sgp.tile([128, 2, CHUNK], BF16, tag="PS")
                        p7 = nc.scalar.activation(
                            PS[:, 0, :], psC[:, :], AF.Exp, scale=-1.0
                        )
                        add_dep_helper(
                            p7.ins, last_ln.ins, sync=False,
                            reason="batch ACT Exp block after Ln",
                        )
                        sg = nc.scalar.activation(
                            PS[:, 1, :], gS1[t][:, :], AF.Exp, scale=-1.0
                        )
                        add_dep_helper(
                            sg.ins, last_ln.ins, sync=False,
                            reason="batch ACT Exp block after Ln",
                        )
                        # Q0 = P7 * sig(z_L); Q1m = (sig - 1) * P7  (w2 odd
                        # block is negated on host to absorb the sign)
                        nc.vector.tensor_tensor(
                            Qt[:, t, 0, :], PS[:, 0, :], PS[:, 1, :], ALU.mult
                        )
                        nc.vector.scalar_tensor_tensor(
                            Qt[:, t, 1, :], PS[:, 1, :], 1.0, PS[:, 0, :],
                            ALU.subtract, ALU.mult,
                        )
                emit_mm2(ci, Qt)
    nc.compile()
    return nc


def _smat7_np():
    S = np.zeros((2, 128, 128), np.float32)
    q7 = np.arange(128)
    for n in range(7):
        node = (2**n - 1) + (q7 >> (7 - n))
        b = (q7 >> (6 - n)) & 1
        S[0, node, q7] = 1.0
        S[1, node, q7] = b
    return S


def _prep_weights(w_d, w_l, n_trees=N_TREES):
    bf16 = ml_dtypes.bfloat16
    w_l = np.asarray(w_l, dtype=np.float32)
    m = w_l.max(axis=-1, keepdims=True)
    e = np.exp(w_l - m, dtype=np.float32)
    sm = e / e.sum(axis=-1, keepdims=True)
    w2fold = (sm[:, 0::2, :] + sm[:, 1::2, :]) * np.float32(1.0 / n_trees)
    w2p = np.empty((n_trees, 2, 128, CLASSES), np.float32)
    w2p[:, 0] = w2fold[:, 0::2, :]
    w2p[:, 1] = -w2fold[:, 1::2, :]
    # permute decision columns: 0..126 internal nodes, 127 pad,
    # 128..255 level-7 nodes; then partition-major [t, p, k, col]
    wd_cols = np.zeros((n_trees, IN_DIM, ND_PAD), np.float32)
    wd_cols[:, :, 0:127] = w_d[:, :, 0:127]
    wd_cols[:, :, 128:256] = w_d[:, :, 127:255]
    wd_p = np.ascontiguousarray(
        wd_cols.reshape(n_trees, KI, 128, ND_PAD).transpose(0, 2, 1, 3)
    )
    return wd_p.astype(bf16), _smat7_np().astype(bf16), w2p.astype(bf16)


last_bass_results = None


def kernel(x, w_d, w_l):
    global last_bass_results
    x = np.asarray(x)
    wd_bf, S7, w2p = _prep_weights(np.asarray(w_d), np.asarray(w_l))
    x_bf = x.astype(ml_dtypes.bfloat16)
    in_maps = []
    for c in range(N_CORES):
        xc = x_bf[c * B_LOC : (c + 1) * B_LOC, :]
        xt = np.ascontiguousarray(xc.T.reshape(KI, 128, B_LOC).transpose(1, 0, 2))
        in_maps.append({"xt": xt, "wd": wd_bf, "smat": S7, "w2": w2p})
    if "nc" not in _CACHE:
        _CACHE["nc"] = _build()
    res = run_bass_kernel_spmd(_CACHE["nc"], in_maps, core_ids=list(range(N_CORES)))
    last_bass_results = res
    return np.concatenate([res.results[c]["out"] for c in range(N_CORES)], axis=0)


# revision 8
# speedup vs baseline: 1.1001x; 1.0530x over previous
"""Trainium2 Bass kernel for nn_DiffForest (soft decision forest forward).

Math: per tree t, z = x @ w_d[t]; p = sigmoid(z); leaf path probs are products
of 8 factors p/(1-p) down a depth-8 tree; output = sum_t leaf_prob @ softmax(w_l[t]) / 10.

Kernel formulation (all on device except small weight prep):
  - The 512 "leaves" come in identical pairs -> fold to 256 paths; fold the
    pair-sum + 1/n_trees into the leaf weight matrix w2 (host, exact).
  - Depth-7 split: path products go to log space only for the first 7 levels:
        C7[q7] = sum_{n<7} softplus(-z_node) + sum_{branch=1} z_node
    which is a matmul with a constant 0/1 matrix S7 [256, 128] (contraction
    [sp(-z); z] over the 127 internal nodes, 2 accumulating matmuls).
    The 8th level is handled elementwise:
        leaf_prob[2*q7]   = exp(-C7[q7]) * sigmoid(z_leaf[q7])
        leaf_prob[2*q7+1] = exp(-C7[q7]) * (1 - sigmoid(z_leaf[q7]))
    with sigmoid(z) = exp(-softplus(-z)) reusing the same Exp/Ln tables.
    This cuts stage-2 PE time 4x vs a full [512, 256] S-matmul.
  - Host permutes w_d columns so cols 0..126 are the internal nodes and cols
    128..255 are the level-7 (leaf-split) nodes, partition-major for
    contiguous DMA. w2 rows are split even/odd with the odd block negated so
    Q1 = (sigmoid - 1) * P7 feeds the leaf matmul directly.
  - Three matmul stages (decision bf16, S7 bf16, leaf bf16), contraction on
    partitions throughout; no on-device transposes.
  - Sharding: data-parallel over batch; each of the 8 cores takes 2048 rows
    of x, weights replicated, no collectives.
  - DMA issue spread across queues: x on Sync, w_d on Scalar (both HWDGE),
    w2/smat/out on GpSimd, so the startup critical path isn't serialized
    behind one queue.
"""

import numpy as np
import ml_dtypes

import concourse.bacc as bacc
import concourse.mybir as mybir
import concourse.tile as tile
from concourse.tile import add_dep_helper
from concourse.bass_utils import run_bass_kernel_spmd

N_CORES = 8
BATCH = 16384
B_LOC = BATCH // N_CORES        # 2048 rows per core
IN_DIM = 2048
N_TREES = 10
ND_PAD = 256                    # decision node columns after permute+pad
CLASSES = 1000
CHUNK = 512                     # batch columns processed per chunk
KI = IN_DIM // 128              # 16 contraction tiles for the decision matmul

BF16 = mybir.dt.bfloat16
F32 = mybir.dt.float32
F16 = mybir.dt.float16
AF = mybir.ActivationFunctionType
ALU = mybir.AluOpType

_CACHE = {}


def _build(b_loc=B_LOC, n_trees=N_TREES):
    n_chunks = b_loc // CHUNK
    nc = bacc.Bacc("TRN2", target_bir_lowering=False)
    xt = nc.dram_tensor("xt", (128, KI, b_loc), BF16, kind="ExternalInput")
    wd = nc.dram_tensor("wd", (n_trees, 128, KI, ND_PAD), BF16, kind="ExternalInput")
    smat = nc.dram_tensor("smat", (2, 128, 128), BF16, kind="ExternalInput")
    w2 = nc.dram_tensor("w2", (n_trees, 2, 128, CLASSES), BF16, kind="ExternalInput")
    out = nc.dram_tensor("out", (b_loc, CLASSES), F32, kind="ExternalOutput")

    with tile.TileContext(nc) as tc:
        with (
            tc.tile_pool(name="const", bufs=1) as constp,
            tc.tile_pool(name="sb", bufs=2) as sb,
            tc.tile_pool(name="wdp", bufs=3) as wdp,
            tc.tile_pool(name="ep", bufs=6) as ep,
            tc.tile_pool(name="gp", bufs=6) as gp,
            tc.tile_pool(name="s1p", bufs=6) as s1p,
            tc.tile_pool(name="sgp", bufs=4) as sgp,
            tc.tile_pool(name="qp", bufs=2) as qp,
            tc.tile_pool(name="outp", bufs=2) as outp,
            tc.tile_pool(name="pz", bufs=2, space="PSUM") as pzp,
            tc.tile_pool(name="pc", bufs=2, space="PSUM") as pcp,
            tc.tile_pool(name="po", bufs=2, space="PSUM") as pop,
        ):
            smat_sb = constp.tile([128, 2, 128], BF16)
            w2_sb = constp.tile([128, n_trees, 2, CLASSES], BF16)

            GROUP = 5
            first_mm = [None]
            tree_mm = {}
            started = False

            def emit_mm2(ci, Qt, last_chunk=False):
                c0 = ci * CHUNK
                for s in range(CHUNK // 128):
                    po = pop.tile([128, 1024], F32, tag="po")
                    osb = outp.tile([128, CLASSES], F32, tag="osb")
                    n_acc = n_trees * 2
                    if last_chunk and s == CHUNK // 128 - 1:
                        # final block: accumulate per column-half so the
                        # first half's copy+store overlaps the second
                        # half's matmuls (shorter tail)
                        for half in range(2):
                            cl = half * 500
                            pl = half * 512
                            i = 0
                            for t in range(n_trees):
                                for lt in range(2):
                                    nc.tensor.matmul(
                                        po[:, pl : pl + 500],
                                        Qt[:, t, lt, s * 128 : (s + 1) * 128],
                                        w2_sb[:, t, lt, cl : cl + 500],
                                        start=(i == 0), stop=(i == n_acc - 1),
                                    )
                                    i += 1
                            nc.vector.tensor_copy(
                                osb[:, cl : cl + 500], po[:, pl : pl + 500]
                            )
                            nc.sync.dma_start(
                                out[c0 + s * 128 : c0 + (s + 1) * 128, cl : cl + 500],
                                osb[:, cl : cl + 500],
                            )
                        continue
                    i = 0
                    for t in range(n_trees):
                        for lt in range(2):
                            first = i == 0
                            last = i == n_acc - 1
                            lhsT = Qt[:, t, lt, s * 128 : (s + 1) * 128]
                            nc.tensor.matmul(
                                po[:, 0:500], lhsT, w2_sb[:, t, lt, 0:500],
                                start=first, stop=last,
                            )
                            nc.tensor.matmul(
                                po[:, 512:1012], lhsT, w2_sb[:, t, lt, 500:1000],
                                start=first, stop=last,
                            )
                            i += 1
                    nc.vector.tensor_copy(osb[:, 0:500], po[:, 0:500])
                    nc.vector.tensor_copy(osb[:, 500:1000], po[:, 512:1012])
                    nc.sync.dma_start(
                        out[c0 + s * 128 : c0 + (s + 1) * 128, :], osb[:, :]
                    )

            for ci in range(n_chunks):
                c0 = ci * CHUNK
                xts = []
                for h in range(2):
                    xp = sb.tile([128, 8, CHUNK], BF16, tag=f"xt{h}")
                    if ci == 0 and h == 0:
                        # sliver the first piece so the first matmul only
                        # waits on 128KB + 64KB of DMA
                        nc.sync.dma_start(xp[:, 0:1, :], xt[:, 0:1, c0 : c0 + CHUNK])
                        nc.sync.dma_start(xp[:, 1:4, :], xt[:, 1:4, c0 : c0 + CHUNK])
                        nc.sync.dma_start(xp[:, 4:8, :], xt[:, 4:8, c0 : c0 + CHUNK])
                    elif ci == 0:
                        nc.sync.dma_start(xp[:, 0:4, :], xt[:, 8:12, c0 : c0 + CHUNK])
                        nc.sync.dma_start(xp[:, 4:8, :], xt[:, 12:16, c0 : c0 + CHUNK])
                    else:
                        xdma = nc.sync.dma_start(
                            xp[:, :, :], xt[:, 8 * h : 8 * (h + 1), c0 : c0 + CHUNK]
                        )
                        # pace: don't let future-chunk x transfers steal HBM
                        # bandwidth from the current chunk's weight stream
                        gate = tree_mm[(ci - 1, 8 if ci == 1 else 1)]
                        add_dep_helper(
                            xdma.ins, gate.ins, sync=True,
                            reason="pace chunk x loads",
                        )
                    xts.append(xp)
                Qt = qp.tile([128, n_trees, 2, CHUNK], BF16, tag="Q")
                for t0 in range(0, n_trees, GROUP):
                    group = list(range(t0, min(t0 + GROUP, n_trees)))
                    gG = {}
                    gE = {}
                    gS1 = {}
                    last_exp = None
                    for t in group:
                        wd_sb = wdp.tile([128, KI, ND_PAD], BF16, tag="wd")
                        if ci == 0 and t == 0:
                            nc.scalar.dma_start(wd_sb[:, 0:1, :], wd[t, :, 0:1, :])
                            nc.scalar.dma_start(wd_sb[:, 1:4, :], wd[t, :, 1:4, :])
                            nc.scalar.dma_start(wd_sb[:, 4:16, :], wd[t, :, 4:16, :])
                        else:
                            wd_dma = nc.scalar.dma_start(wd_sb[:, :, :], wd[t, :, :, :])
                            if ci == 0 and t in (1, 2):
                                add_dep_helper(
                                    wd_dma.ins, first_mm[0].ins, sync=True,
                                    reason="startup: critical pieces first",
                                )
                        G = gp.tile([128, 2, CHUNK], BF16, tag="G")
                        E = ep.tile([128, 2, CHUNK], F16, tag="E")
                        S1 = s1p.tile([128, CHUNK], BF16, tag="S1")
                        gG[t] = G
                        gE[t] = E
                        gS1[t] = S1
                        for dt_ in range(2):
                            psz = pzp.tile([128, CHUNK], F32, tag="psz")
                            for k in range(KI):
                                lhsT = wd_sb[:, k, dt_ * 128 : (dt_ + 1) * 128]
                                mm = nc.tensor.matmul(
                                    psz[:, :], lhsT, xts[k // 8][:, k % 8, :],
                                    start=(k == 0), stop=(k == KI - 1),
                                )
                                if first_mm[0] is None:
                                    first_mm[0] = mm
                                if k == 0 and dt_ == 0:
                                    tree_mm[(ci, t)] = mm
                            last_exp = nc.scalar.activation(
                                E[:, dt_, :], psz[:, :], AF.Exp, scale=-1.0
                            )
                            if dt_ == 0:
                                nc.vector.tensor_copy(G[:, 1, :], psz[:, :])
                    # softplus(-z) = ln(exp(-z)+1); gate Lns on the group's
                    # last Exp to batch ACT table sets
                    last_ln = None
                    for t in group:
                        ln0 = nc.scalar.activation(
                            gG[t][:, 0, :], gE[t][:, 0, :], AF.Ln, bias=1.0
                        )
                        add_dep_helper(
                            ln0.ins, last_exp.ins, sync=False,
                            reason="batch ACT Ln block after Exp",
                        )
                        ln1 = nc.scalar.activation(
                            gS1[t][:, :], gE[t][:, 1, :], AF.Ln, bias=1.0
                        )
                        add_dep_helper(
                            ln1.ins, last_exp.ins, sync=False,
                            reason="batch ACT Ln block after Exp",
                        )
                        last_ln = ln1
                    if not started:
                        nc.gpsimd.dma_start(
                            smat_sb[:, :, :],
                            smat[:, :, :].rearrange("j p q -> p j q"),
                        )
                        started = True
                    if ci == 0 and t0 == GROUP:
                        # defer the 5MB w2 transfer until the startup weight
                        # stream (trees 0..4) is through, else it starves
                        # the PE of decision weights; it only needs to land
                        # before the first leaf matmul (~80us in)
                        gate = tree_mm[(0, GROUP)]
                        for t in range(n_trees):
                            wdma = nc.gpsimd.dma_start(
                                w2_sb[:, t, :, :],
                                w2[t, :, :, :].rearrange("l p c -> p l c"),
                            )
                            add_dep_helper(
                                wdma.ins, gate.ins, sync=True,
                                reason="defer w2 load past startup",
                            )
                    for t in group:
                        psC = pcp.tile([128, CHUNK], F32, tag="psC")
                        nc.tensor.matmul(
                            psC[:, :], smat_sb[:, 0, :], gG[t][:, 0, :],
                            start=True, stop=False,
                        )
                        nc.tensor.matmul(
                            psC[:, :], smat_sb[:, 1, :], gG[t][:, 1, :],
                            start=False, stop=True,
                        )
                        PS = sgp.tile([128, 2, CHUNK], BF16, tag="PS")
                        p7 = nc.scalar.activation(
                            PS[:, 0, :], psC[:, :], AF.Exp, scale=-1.0
                        )
                        add_dep_helper(
                            p7.ins, last_ln.ins, sync=False,
                            reason="batch ACT Exp block after Ln",
                        )
                        sg = nc.scalar.activation(
                            PS[:, 1, :], gS1[t][:, :], AF.Exp, scale=-1.0
                        )
                        add_dep_helper(
                            sg.ins, last_ln.ins, sync=False,
                            reason="batch ACT Exp block after Ln",
                        )
                        # Q0 = P7 * sig(z_L); Q1m = (sig - 1) * P7  (w2 odd
                        # block is negated on host to absorb the sign)
                        nc.vector.tensor_tensor(
                            Qt[:, t, 0, :], PS[:, 0, :], PS[:, 1, :], ALU.mult
                        )
                        nc.vector.scalar_tensor_tensor(
                            Qt[:, t, 1, :], PS[:, 1, :], 1.0, PS[:, 0, :],
                            ALU.subtract, ALU.mult,
                        )
                emit_mm2(ci, Qt, last_chunk=(ci == n_chunks - 1))
    nc.compile()
    return nc


def _smat7_np():
    S = np.zeros((2, 128, 128), np.float32)
    q7 = np.arange(128)
    for n in range(7):
        node = (2**n - 1) + (q7 >> (7 - n))
        b = (q7 >> (6 - n)) & 1
        S[0, node, q7] = 1.0
        S[1, node, q7] = b
    return S


def _prep_weights(w_d, w_l, n_trees=N_TREES):
    bf16 = ml_dtypes.bfloat16
    w_l = np.asarray(w_l, dtype=np.float32)
    m = w_l.max(axis=-1, keepdims=True)
    e = np.exp(w_l - m, dtype=np.float32)
    sm = e / e.sum(axis=-1, keepdims=True)
    w2fold = (sm[:, 0::2, :] + sm[:, 1::2, :]) * np.float32(1.0 / n_trees)
    w2p = np.empty((n_trees, 2, 128, CLASSES), np.float32)
    w2p[:, 0] = w2fold[:, 0::2, :]
    w2p[:, 1] = -w2fold[:, 1::2, :]
    # permute decision columns: 0..126 internal nodes, 127 pad,
    # 128..255 level-7 nodes; then partition-major [t, p, k, col]
    wd_cols = np.zeros((n_trees, IN_DIM, ND_PAD), np.float32)
    wd_cols[:, :, 0:127] = w_d[:, :, 0:127]
    wd_cols[:, :, 128:256] = w_d[:, :, 127:255]
    wd_p = np.ascontiguousarray(
        wd_cols.reshape(n_trees, KI, 128, ND_PAD).transpose(0, 2, 1, 3)
    )
    return wd_p.astype(bf16), _smat7_np().astype(bf16), w2p.astype(bf16)


last_bass_results = None


def kernel(x, w_d, w_l):
    global last_bass_results
    x = np.asarray(x)
    wd_bf, S7, w2p = _prep_weights(np.asarray(w_d), np.asarray(w_l))
    x_bf = x.astype(ml_dtypes.bfloat16)
    in_maps = []
    for c in range(N_CORES):
        xc = x_bf[c * B_LOC : (c + 1) * B_LOC, :]
        xt = np.ascontiguousarray(xc.T.reshape(KI, 128, B_LOC).transpose(1, 0, 2))
        in_maps.append({"xt": xt, "wd": wd_bf, "smat": S7, "w2": w2p})
    if "nc" not in _CACHE:
        _CACHE["nc"] = _build()
    res = run_bass_kernel_spmd(_CACHE["nc"], in_maps, core_ids=list(range(N_CORES)))
    last_bass_results = res
    return np.concatenate([res.results[c]["out"] for c in range(N_CORES)], axis=0)
